# revision 16
# baseline (speedup 1.0000x reference)
"""Trainium2 Bass kernel for CustomConvolution2d.

Problem: y = conv2d(x, weight, stride=1, pad=1) + bias
  x: [32, 64, 128, 128] f32, weight: [64, 64, 3, 3] f32, bias: [64] f32.

Strategy (data-parallel, batch/8 = 4 images per core):

Per image, x is host-padded to [64, 130, 130] and loaded into SBUF
partitions 0-63; partitions 64-127 hold the same buffer shifted +1 row
(on-chip SBUF->SBUF copy). A matmul whose rhs spans partitions
(e, ci) = (row-shift, channel) then sees x rows r and r+1 at once, so
K = 128 is fully used.

The 3x3 conv over output rows r0..r0+3 (free dim N = 4 rows x 128 cols
= 512) is 3 matmuls (one per kw), accumulating in PSUM, with lhsT

    [[W(kh=1,kw), W(kh=0,kw)],
     [W(kh=2,kw),     0     ]]   (K blocks = e, M blocks = d)

so PSUM partitions 0-63  (P0) get the kh=1,2 taps of rows r0+j, and
partitions 64-127 (P1) get the kh=0 tap of rows r0+j+1. The final
output row h = P0(j=h-r0) + P1(j=h-r0-1) + bias is produced by a single
fused DVE scalar_tensor_tensor per region (add bias, add the two PSUM
halves lane-wise across partition bases 0 and 64). 12 of the 16 lhsT
quadrants are useful -> 75% PE utilization at K=M=128, N=512.

Weights stream at 1 cycle/row via float32r (N=512 >= 256).
"""

import numpy as np

N_FULL = 32
C = 64
H = 128
W = 128
HP = H + 2  # 130
NCORES = 8
NPER = N_FULL // NCORES  # 4 images per core

_cache = {}


def _build(dt_name: str):
    """Build the Bass program once per dtype. Returns the Bass object."""
    import concourse.bass as bass
    import concourse.tile as tile
    from concourse import bacc, mybir

    DT = getattr(mybir.dt, dt_name)
    F32 = mybir.dt.float32
    IDENT = mybir.ActivationFunctionType.Identity

    nc = bacc.Bacc(trn_type="TRN2", target_bir_lowering=False, debug=False,
                   num_devices=NCORES)

    xp = nc.dram_tensor("xp", [NPER, C, HP, HP], DT, kind="ExternalInput").ap()
    wpack = nc.dram_tensor("wpack", [3, 128, 128], DT, kind="ExternalInput").ap()
    biasb = nc.dram_tensor("biasb", [C, 1], F32, kind="ExternalInput").ap()
    out = nc.dram_tensor("out", [NPER, C, H, W], F32, kind="ExternalOutput").ap()

    with tile.TileContext(nc) as tc:
        with (
            tc.tile_pool(name="wpool", bufs=1) as wpool,
            tc.tile_pool(name="cpool", bufs=1) as cpool,
            tc.tile_pool(name="xpool", bufs=2) as xpool,
            tc.tile_pool(name="opool", bufs=2) as opool,
            tc.tile_pool(name="ppool", bufs=1, space="PSUM") as ppool,
            tc.tile_pool(name="dpool", bufs=1, space="PSUM") as dpool,
        ):
            wk = []
            for k in range(3):
                wt = wpool.tile([128, 128], DT, name=f"wk{k}")
                nc.sync.dma_start(out=wt[:, :], in_=wpack[k])
                wk.append(wt)
            bias_sb = cpool.tile([C, 1], F32)
            nc.sync.dma_start(out=bias_sb[:, :], in_=biasb[:, :])

            # Persistent PSUM accumulators (6 banks).  Rewriting the same
            # tile produces only the WAR wait on its last reader (DVE),
            # keeping fp32r matmuls at <=1 sync wait (the fused LW struct
            # has a single EVENTS slot).
            pt = [ppool.tile([128, 512], F32, name=f"pacc{i}") for i in
                  range(6)]

            # Dummy matmuls: let PE observe each weight-DMA semaphore here
            # (one lane per matmul) so real matmuls never wait on them.
            # fp32r requires full 128-column tiling and even innermost count,
            # so dummies are M=128, N=2.
            pdummy = dpool.tile([128, 2], F32)
            for k in range(3):
                nc.tensor.matmul(pdummy[:, 0:2], wk[k][:, :],
                                 wk[k][:, 0:2], start=True, stop=True)

            for n in range(NPER):
                x2 = xpool.tile([128, HP, HP], DT, name="x2")
                nc.sync.dma_start(out=x2[0:64, :, :], in_=xp[n])
                # partitions 64-127 = same image shifted +1 row
                nc.sync.dma_start(out=x2[64:128, 0:HP - 1, :],
                                  in_=x2[0:64, 1:HP, :])
                # dummy matmuls absorb the x-load + dup DMA waits for PE
                # (one DMA semaphore each, keeping real matmuls at <=1 wait)
                nc.tensor.matmul(pdummy[:, 0:2], wk[0][0:64, :],
                                 x2[0:64, 0:1, 0:2], start=True, stop=True)
                nc.tensor.matmul(pdummy[:, 0:2], wk[0][:, :],
                                 x2[:, 0:1, 0:2], start=True, stop=True)

                pprev = None
                for c in range(4):  # output row chunks of 32
                    osb = opool.tile([C, 4096], F32, name="osb")
                    for b8 in range(8):  # blocks of 4 output rows
                        b = c * 8 + b8
                        r0 = 4 * b
                        p = pt[(n * 32 + b) % 6]
                        for k in range(3):
                            nc.tensor.matmul(
                                p[:, :], wk[k][:, :],
                                x2[:, r0 + 1:r0 + 5, k:k + 128],
                                start=(k == 0), stop=(k == 2))
                        o0 = b8 * 512
                        # DVE is the only PSUM reader (keeps the k==0 matmul
                        # at 2 waits: DVE + PE), and each DVE op touches at
                        # most one PSUM operand (DVE has one PSUM read port).
                        nc.vector.tensor_scalar_add(
                            osb[:, o0:o0 + 512], p[0:64, :], bias_sb[:, 0:1])
                        nc.vector.tensor_add(
                            osb[:, o0 + 128:o0 + 512],
                            osb[:, o0 + 128:o0 + 512], p[64:128, 0:384])
                        if pprev is not None:
                            nc.vector.tensor_add(
                                osb[:, o0:o0 + 128],
                                osb[:, o0:o0 + 128], pprev[64:128, 384:512])
                        pprev = p
                    nc.sync.dma_start(out=out[n, :, 32 * c:32 * c + 32, :],
                                      in_=osb[:, :])
    nc.compile()
    return nc


def _get_nc(dt_name: str):
    if dt_name not in _cache:
        _cache[dt_name] = _build(dt_name)
    return _cache[dt_name]


_last_results = None


def prep_in_maps(x, weight, bias, dt_name="float32r"):
    """Host prep: pad x, pack lhsT weights, build per-core input maps."""
    x = np.ascontiguousarray(np.asarray(x), dtype=np.float32)
    weight = np.asarray(weight, dtype=np.float32)
    bias = np.asarray(bias, dtype=np.float32)

    if dt_name == "bfloat16":
        import ml_dtypes
        np_dt = ml_dtypes.bfloat16
    else:
        np_dt = np.float32

    # host prep: zero-pad x spatially
    xp = np.zeros((N_FULL, C, HP, HP), dtype=np_dt)
    xp[:, :, 1:HP - 1, 1:HP - 1] = x

    # lhsT pack: wpack[kw][e*64+ci, d*64+co]
    #   (e=0,d=0)=W[co,ci,1,kw]  (e=0,d=1)=W[co,ci,0,kw]
    #   (e=1,d=0)=W[co,ci,2,kw]  (e=1,d=1)=0
    wt = weight.transpose(1, 0, 2, 3)  # [ci, co, kh, kw]
    wpack = np.zeros((3, 128, 128), dtype=np_dt)
    for k in range(3):
        wpack[k, 0:64, 0:64] = wt[:, :, 1, k]
        wpack[k, 0:64, 64:128] = wt[:, :, 0, k]
        wpack[k, 64:128, 0:64] = wt[:, :, 2, k]

    biasb = bias.reshape(C, 1).astype(np.float32)

    nc = _get_nc(dt_name)
    in_maps = [
        {"xp": xp[c * NPER:(c + 1) * NPER], "wpack": wpack, "biasb": biasb}
        for c in range(NCORES)
    ]
    return in_maps, nc


def kernel(x, weight, bias, dt_name="float32r", trace=False):
    global _last_results
    from concourse import bass_utils

    in_maps, nc = prep_in_maps(x, weight, bias, dt_name)
    res = bass_utils.run_bass_kernel_spmd(nc, in_maps, list(range(NCORES)),
                                          trace=trace)
    _last_results = res
    return np.concatenate([res.results[c]["out"] for c in range(NCORES)],
                          axis=0)
